# revision 1
# baseline (speedup 1.0000x reference)
"""Mistral MoE layer (H=2048, F=8192, E=8, top-2) on 8 Trainium2 NeuronCores.

Strategy (expert parallelism, per the sharding hint):
  - Host computes the (tiny) gate: logits = x @ gate_w, top-2, softmax.
    This is 0.004% of the FLOPs; the expert FFNs dominate.
  - Host "all-to-all dispatch": tokens are gathered per expert into a
    padded capacity buffer of C tokens (C = max expert load rounded up).
  - Core e runs expert e's SwiGLU FFN over its C tokens:
        y = (silu(x @ w1) * (x @ w3)) @ w2, scaled per-token by the
    combined gate weight.
  - Host "combine": scatter-add each expert's rows back into the output.

Device kernel (per core):
  Stage 1 computes hT/uT = w1/w3-projections in *transposed* form
  [F, C] so stage 2 can use them directly as the stationary matmul
  operand without any on-device transpose.  Matmuls run as float32r
  (FP22 truncated fp32 — full 78.6 TF/s PE rate at moving dim >= 256).
  yT = silu(hT) * uT is stored bf16; stage 2 (down-projection) runs
  bf16 x bf16.  F is processed in groups of G=8 f-tiles; each group's
  partial down-projection output is accumulated via a DRAM bounce
  buffer: out_g = psum * gate_w + out_{g-1} (fused DVE op).
"""

import math
import os

import numpy as np
import ml_dtypes

import concourse.bass as bass
import concourse.mybir as mybir
import concourse.tile as tile
from concourse import bacc
from concourse.bass_utils import run_bass_kernel_spmd

P = 128
H = 2048
F = 8192
E = 8
TOP_K = 2

_kernel_cache: dict = {}

# Test-harness knobs (ignored in normal use): when TRACE is true, the SPMD
# run captures an NTFF profile and the BassKernelResults lands in LAST_RESULT.
TRACE = False
LAST_RESULT = None


def build_expert_kernel(C, H_=H, F_=F, c_chunk=384, ho_chunk=512, G=8):
    """One expert's SwiGLU FFN over C tokens; returns finalized Bacc."""
    f32 = mybir.dt.float32
    f32r = mybir.dt.float32r
    bf16 = mybir.dt.bfloat16

    n_hh = H_ // P          # contraction tiles over hidden dim (stage 1)
    n_f = F_ // P           # f tiles
    n_ct = C // P           # token tiles (stage 2 output partitions)
    n_cch = C // c_chunk    # moving-dim chunks over tokens (stage 1)
    n_ho = H_ // ho_chunk   # moving-dim chunks over hidden dim (stage 2)
    n_groups = n_f // G
    assert C % c_chunk == 0 and C % P == 0 and F_ % (G * P) == 0

    nc = bacc.Bacc("TRN2", target_bir_lowering=False, debug=False)
    xt_d = nc.dram_tensor("xt", [H_, C], f32r, kind="ExternalInput")
    w1_d = nc.dram_tensor("w1", [H_, F_], f32r, kind="ExternalInput")
    w3_d = nc.dram_tensor("w3", [H_, F_], f32r, kind="ExternalInput")
    w2_d = nc.dram_tensor("w2b", [F_, H_], bf16, kind="ExternalInput")
    gw_d = nc.dram_tensor("gws", [P, n_ct], f32, kind="ExternalInput")
    out_d = nc.dram_tensor("out", [C, H_], f32, kind="ExternalOutput")

    xt_r = xt_d[:, :].rearrange("(ho hi) c -> hi ho c", hi=P)
    w1_r = w1_d[:, :].rearrange("(ho hi) f -> hi ho f", hi=P)
    w3_r = w3_d[:, :].rearrange("(ho hi) f -> hi ho f", hi=P)

    with tile.TileContext(nc) as tc:
        with (
            tc.tile_pool(name="persist", bufs=1) as persist,
            tc.tile_pool(name="wpool", bufs=2) as wpool,
            tc.tile_pool(name="ypool", bufs=1) as ypool,
            tc.tile_pool(name="spool", bufs=2) as spool,
            tc.tile_pool(name="opool", bufs=2) as opool,
            tc.tile_pool(name="psum", bufs=1, space="PSUM") as psum,
            tc.tile_pool(name="dram", bufs=1, space="DRAM") as dram,
        ):
            # first f-tile's weights are issued before xt so the PE can start
            # as soon as xt lands; xt DMAs alternate across the two HWDGE
            # issuing engines (sync, scalar) to halve serial issue time.
            w1_first = wpool.tile([P, n_hh, P], f32r, tag="w1t", name="w1_first")
            nc.sync.dma_start(w1_first[:], w1_r[:, :, bass.ts(0, P)])
            w3_first = wpool.tile([P, n_hh, P], f32r, tag="w3t", name="w3_first")
            nc.scalar.dma_start(w3_first[:], w3_r[:, :, bass.ts(0, P)])
            xt_s = persist.tile([P, n_hh, C], f32r, name="xt_s")
            for hh in range(n_hh):
                eng = nc.sync if hh % 2 == 0 else nc.scalar
                eng.dma_start(xt_s[:, hh, :], xt_r[:, hh, :])
            gw_s = persist.tile([P, n_ct], f32, name="gw_s")
            nc.scalar.dma_start(gw_s[:], gw_d[:, :])
            part_d = dram.tile([C, H_], f32, name="part")

            for g in range(n_groups):
                # ---- stage 1: yT[f_tile, :] for the G f-tiles of this group
                yt = ypool.tile([P, G, C], bf16, tag="yt", name="yt")
                for fi in range(G):
                    f = g * G + fi
                    fsl = bass.ts(f, P)
                    if g == 0 and fi == 0:
                        w1_t, w3_t = w1_first, w3_first
                    else:
                        w1_t = wpool.tile([P, n_hh, P], f32r, tag="w1t", name="w1_t")
                        nc.sync.dma_start(w1_t[:], w1_r[:, :, fsl])
                        w3_t = wpool.tile([P, n_hh, P], f32r, tag="w3t", name="w3_t")
                        nc.sync.dma_start(w3_t[:], w3_r[:, :, fsl])
                    for ci in range(n_cch):
                        csl = bass.ts(ci, c_chunk)
                        ph = psum.tile([P, c_chunk], f32, tag="ph", bufs=2, name="ph")
                        pu = psum.tile([P, c_chunk], f32, tag="pu", bufs=2, name="pu")
                        for hh in range(n_hh):
                            nc.tensor.matmul(
                                ph[:],
                                w1_t[:, hh, :],
                                xt_s[:, hh, csl],
                                start=(hh == 0),
                                stop=(hh == n_hh - 1),
                            )
                        for hh in range(n_hh):
                            nc.tensor.matmul(
                                pu[:],
                                w3_t[:, hh, :],
                                xt_s[:, hh, csl],
                                start=(hh == 0),
                                stop=(hh == n_hh - 1),
                            )
                        sl = spool.tile([P, c_chunk], f32, tag="sl", name="sl")
                        nc.scalar.activation(
                            sl[:], ph[:], mybir.ActivationFunctionType.Silu
                        )
                        nc.vector.tensor_tensor(
                            yt[:, fi, csl], sl[:], pu[:], mybir.AluOpType.mult
                        )

                # ---- stage 2: partial down-projection for this group
                w2_t = wpool.tile([P, G, H_], bf16, tag="w2t", bufs=1, name="w2_t")
                for fi in range(G):
                    f = g * G + fi
                    nc.sync.dma_start(w2_t[:, fi, :], w2_d[bass.ts(f, P), :])
                # two ho-half passes so the PSUM tile is 2 banks and can be
                # double-buffered (next ct's matmuls overlap this ct's DVE read)
                n_half = n_ho // 2 if n_ho >= 2 else 1
                half_w = n_half * ho_chunk
                for ct in range(n_ct):
                    ctsl = bass.ts(ct, P)
                    ot = opool.tile([P, H_], f32, tag="ot", name="ot")
                    gsl = gw_s[:, ct : ct + 1]
                    prev = None
                    if g > 0:
                        prev = opool.tile([P, H_], f32, tag="prev", name="prev")
                        nc.sync.dma_start(prev[:], part_d[ctsl, :])
                    for hf in range(n_ho // n_half):
                        po = psum.tile([P, half_w], f32, tag="po", bufs=2, name="po")
                        for fi in range(G):
                            for ho in range(n_half):
                                hosl = bass.ts(hf * n_half + ho, ho_chunk)
                                nc.tensor.matmul(
                                    po[:, bass.ts(ho, ho_chunk)],
                                    yt[:, fi, ctsl],
                                    w2_t[:, fi, hosl],
                                    start=(fi == 0),
                                    stop=(fi == G - 1),
                                )
                        hsl = bass.ts(hf, half_w)
                        if g == 0:
                            nc.vector.tensor_scalar_mul(ot[:, hsl], po[:], gsl)
                        else:
                            nc.vector.scalar_tensor_tensor(
                                ot[:, hsl],
                                po[:],
                                gsl,
                                prev[:, hsl],
                                mybir.AluOpType.mult,
                                mybir.AluOpType.add,
                            )
                    dst = out_d if g == n_groups - 1 else part_d
                    nc.sync.dma_start(dst[ctsl, :], ot[:])
    nc.finalize()
    return nc


def _route(x, gate_w):
    """Host gate: top-2 + softmax.  Returns (idx per expert, weight per expert)."""
    xs = x.reshape(-1, x.shape[-1])
    logits = xs.astype(np.float32) @ gate_w.astype(np.float32)  # [T, E]
    # top-2 (ties broken by lower index, matching jax.lax.top_k)
    e1 = np.argmax(logits, axis=1)
    l1 = logits[np.arange(len(logits)), e1]
    masked = logits.copy()
    masked[np.arange(len(logits)), e1] = -np.inf
    e2 = np.argmax(masked, axis=1)
    l2 = masked[np.arange(len(logits)), e2]
    # softmax over the two logits
    w_hi = 1.0 / (1.0 + np.exp(l2 - l1))
    w_lo = 1.0 - w_hi
    idxs, gws = [], []
    for e in range(E):
        sel1 = e1 == e
        sel2 = e2 == e
        idx = np.nonzero(sel1 | sel2)[0]
        w = np.where(sel1[idx], w_hi[idx], w_lo[idx]).astype(np.float32)
        idxs.append(idx)
        gws.append(w)
    return xs, idxs, gws


def kernel(x, gate_w, w1, w3, w2):
    x = np.asarray(x)
    gate_w = np.asarray(gate_w)
    w1 = np.asarray(w1)
    w3 = np.asarray(w3)
    w2 = np.asarray(w2)

    xs, idxs, gws = _route(x, gate_w)
    T = xs.shape[0]
    max_load = max(len(i) for i in idxs)
    # SBUF budget supports capacities up to 1152 tokens/core; rarer, more
    # imbalanced routings are handled by running the same NEFF multiple
    # times over token chunks of <= CAP per expert.
    CAP = int(os.environ.get("MOE_CAP", "1152"))
    C = min(CAP, max(384, int(math.ceil(max_load / 384.0)) * 384))
    n_pass = int(math.ceil(max_load / float(C)))

    key = ("k", C)
    if key not in _kernel_cache:
        _kernel_cache[key] = build_expert_kernel(C)
    nc = _kernel_cache[key]

    w_maps = [
        {
            "w1": np.ascontiguousarray(w1[e], dtype=np.float32),
            "w3": np.ascontiguousarray(w3[e], dtype=np.float32),
            "w2b": np.ascontiguousarray(w2[e]).astype(ml_dtypes.bfloat16),
        }
        for e in range(E)
    ]

    global LAST_RESULT
    out_flat = np.zeros((T, H), np.float32)
    for p in range(n_pass):
        in_maps = []
        p_idx = []
        for e in range(E):
            idx = idxs[e][p * C : (p + 1) * C]
            gw = gws[e][p * C : (p + 1) * C]
            n_e = len(idx)
            p_idx.append(idx)
            xt = np.zeros((H, C), np.float32)
            if n_e:
                xt[:, :n_e] = xs[idx].T
            gwpad = np.zeros(C, np.float32)
            gwpad[:n_e] = gw
            in_maps.append(
                {
                    "xt": xt,
                    "gws": np.ascontiguousarray(gwpad.reshape(C // P, P).T),
                    **w_maps[e],
                }
            )
        if TRACE:
            try:
                res = run_bass_kernel_spmd(
                    nc,
                    in_maps,
                    core_ids=list(range(E)),
                    trace=True,
                    trace_cores=list(range(E)),
                )
            except Exception as exc:
                import traceback

                print("TRACE FAILED:", exc)
                traceback.print_exc()
                res = run_bass_kernel_spmd(nc, in_maps, core_ids=list(range(E)))
        else:
            res = run_bass_kernel_spmd(nc, in_maps, core_ids=list(range(E)))
        LAST_RESULT = res
        for e in range(E):
            n_e = len(p_idx[e])
            if n_e:
                out_flat[p_idx[e]] += res.results[e]["out"][:n_e]
    return out_flat.reshape(x.shape)



# revision 4
# speedup vs baseline: 1.1443x; 1.1443x over previous
"""Mistral MoE layer (H=2048, F=8192, E=8, top-2) on 8 Trainium2 NeuronCores.

Strategy: tensor-parallel over the expert FFN intermediate dim (F-shard).
Each core owns a 1024-wide slice of F for ALL 8 experts and processes,
sequentially per expert, exactly the tokens routed to that expert:

  stage 1:  hT[f, c] = w1_slice.T x ;  uT[f, c] = w3_slice.T x
            yT[f, c] = silu(hT) * uT                     (bf16)
  stage 2:  partial_out[h, c] = w2_slice.T yT, scaled by the combined
            gate weight per token (folded into the PSUM-evacuation op)

The host sums the 8 per-core partial outputs and scatter-adds them into
the token-major output.  This gives perfect load balance (every core does
sum(L_e)/8 = 1024 token-pair-equivalents regardless of routing skew), no
capacity padding (moving dim = tokens, chunked to the real count), no
DRAM bounce accumulation (stage-2 contraction is only 8 f-tiles -> a
single PSUM accumulation group), and all-bf16 matmuls (rel err ~4e-3,
measured offline, vs the 2e-2 gate).
"""

import math

import numpy as np
import ml_dtypes

import concourse.bass as bass
import concourse.mybir as mybir
import concourse.tile as tile
from concourse import bacc
from concourse.bass_utils import run_bass_kernel_spmd

P = 128
H = 2048
F = 8192
E = 8
TOP_K = 2
N_CORES = 8
F_LOC = F // N_CORES          # 1024 — per-core F slice
N_FI = F_LOC // P             # 8 f-tiles per expert per core
N_HH = H // P                 # 16 contraction tiles over hidden dim
SEG_CAP = 1280                # max padded tokens per segment (SBUF budget)

_kernel_cache: dict = {}

# Test-harness knobs: when TRACE is true the SPMD run captures an NTFF
# profile and the BassKernelResults lands in LAST_RESULT.
TRACE = False
LAST_RESULT = None


def _chunks(L):
    """Split L tokens into matmul moving-dim chunks <= 512, multiple of 4."""
    n = max(1, math.ceil(L / 512))
    out = []
    rem = L
    for i in range(n):
        c = (rem // (n - i) + 3) // 4 * 4
        c = min(c, rem)
        out.append(c)
        rem -= c
    assert sum(out) == L and all(c <= 512 for c in out)
    return out


def build_kernel(seg_lens):
    """One core's program: per segment s (expert slot) of seg_lens[s] padded
    tokens, run the F-sliced SwiGLU FFN.  Returns finalized Bacc."""
    f32 = mybir.dt.float32
    bf16 = mybir.dt.bfloat16
    nseg = len(seg_lens)
    L_tot = sum(seg_lens)
    offs = np.concatenate([[0], np.cumsum(seg_lens)]).astype(int)

    nc = bacc.Bacc("TRN2", target_bir_lowering=False, debug=False)
    xt_d = nc.dram_tensor("xt", [H, L_tot], bf16, kind="ExternalInput")
    gw_d = nc.dram_tensor("gwb", [P, L_tot], f32, kind="ExternalInput")
    w1_d = nc.dram_tensor("w1s", [nseg * N_FI * P, H], bf16, kind="ExternalInput")
    w3_d = nc.dram_tensor("w3s", [nseg * N_FI * P, H], bf16, kind="ExternalInput")
    w2_d = nc.dram_tensor("w2s", [nseg * N_FI * P, H], bf16, kind="ExternalInput")
    out_d = nc.dram_tensor("out", [H, L_tot], f32, kind="ExternalOutput")

    xt_r = xt_d[:, :].rearrange("(ho hi) c -> hi ho c", hi=P)
    out_r = out_d[:, :].rearrange("(ht hp) c -> hp ht c", hp=P)

    with tile.TileContext(nc) as tc:
        with (
            tc.tile_pool(name="xpool", bufs=2) as xpool,
            tc.tile_pool(name="gpool", bufs=2) as gpool,
            tc.tile_pool(name="wpool", bufs=2) as wpool,
            tc.tile_pool(name="w2pool", bufs=1) as w2pool,
            tc.tile_pool(name="ypool", bufs=2) as ypool,
            tc.tile_pool(name="spool", bufs=2) as spool,
            tc.tile_pool(name="opool", bufs=2) as opool,
            tc.tile_pool(name="psum", bufs=1, space="PSUM") as psum,
        ):
            # Prefetched token/gate tiles, one segment ahead (so the DMA for
            # segment si+1 is enqueued before stage-2(si)'s output DMAs fill
            # the FIFO queues).
            xt_tiles: dict = {}
            gw_tiles: dict = {}

            def fetch_seg(si):
                L = seg_lens[si]
                o = int(offs[si])
                xt_s = xpool.tile([P, N_HH, L], bf16, tag="xt", name=f"xt{si}")
                nc.sync.dma_start(xt_s[:, 0:8, :], xt_r[:, 0:8, o : o + L])
                nc.scalar.dma_start(xt_s[:, 8:16, :], xt_r[:, 8:16, o : o + L])
                gw_s = gpool.tile([P, L], f32, tag="gw", name=f"gw{si}")
                nc.scalar.dma_start(gw_s[:], gw_d[:, o : o + L])
                xt_tiles[si] = xt_s
                gw_tiles[si] = gw_s

            # first segment's first weight tiles go out before its tokens so
            # the PE can start as soon as the tokens land
            w1_first = wpool.tile([P, N_HH, P], bf16, tag="w1t", name="w1_first")
            nc.sync.dma_start(
                w1_first[:], w1_d[bass.ts(0, P), :].rearrange("p (ho f) -> p ho f", f=P)
            )
            w3_first = wpool.tile([P, N_HH, P], bf16, tag="w3t", name="w3_first")
            nc.scalar.dma_start(
                w3_first[:], w3_d[bass.ts(0, P), :].rearrange("p (ho f) -> p ho f", f=P)
            )
            fetch_seg(0)

            for si in range(nseg):
                L = seg_lens[si]
                o = int(offs[si])
                ch = _chunks(L)
                xt_s = xt_tiles.pop(si)
                gw_s = gw_tiles.pop(si)

                # ---- stage 1: yT[f, c] for the 8 f-tiles of this segment
                yt = ypool.tile([P, N_FI, L], bf16, tag="yt", name=f"yt{si}")
                for fi in range(N_FI):
                    row = bass.ts(si * N_FI + fi, P)
                    if si == 0 and fi == 0:
                        w1_t, w3_t = w1_first, w3_first
                    else:
                        w1_t = wpool.tile([P, N_HH, P], bf16, tag="w1t", name="w1_t")
                        nc.sync.dma_start(
                            w1_t[:], w1_d[row, :].rearrange("p (ho f) -> p ho f", f=P)
                        )
                        w3_t = wpool.tile([P, N_HH, P], bf16, tag="w3t", name="w3_t")
                        nc.scalar.dma_start(
                            w3_t[:], w3_d[row, :].rearrange("p (ho f) -> p ho f", f=P)
                        )
                    c0 = 0
                    for cw in ch:
                        csl = slice(c0, c0 + cw)
                        ph = psum.tile([P, cw], f32, tag="ph", bufs=2, name="ph")
                        for hh in range(N_HH):
                            nc.tensor.matmul(
                                ph[:],
                                w1_t[:, hh, :],
                                xt_s[:, hh, csl],
                                start=(hh == 0),
                                stop=(hh == N_HH - 1),
                            )
                        pu = psum.tile([P, cw], f32, tag="pu", bufs=2, name="pu")
                        for hh in range(N_HH):
                            nc.tensor.matmul(
                                pu[:],
                                w3_t[:, hh, :],
                                xt_s[:, hh, csl],
                                start=(hh == 0),
                                stop=(hh == N_HH - 1),
                            )
                        sl = spool.tile([P, cw], f32, tag="sl", name="sl")
                        nc.scalar.activation(
                            sl[:], ph[:], mybir.ActivationFunctionType.Silu
                        )
                        nc.vector.tensor_tensor(
                            yt[:, fi, csl], sl[:], pu[:], mybir.AluOpType.mult
                        )
                        c0 += cw

                # prefetch next segment's tokens/gates ahead of the out-DMA flood
                if si + 1 < nseg:
                    fetch_seg(si + 1)

                # ---- stage 2: partial down-projection, gate-scaled
                w2_t = w2pool.tile([P, N_FI, H], bf16, tag="w2t", name="w2_t")
                nc.scalar.dma_start(
                    w2_t[:],
                    w2_d[bass.ts(si, N_FI * P), :].rearrange("(f p) h -> p f h", p=P),
                )
                for ht in range(H // P):
                    ot = opool.tile([P, L], f32, tag="ot", name="ot")
                    c0 = 0
                    for cw in ch:
                        csl = slice(c0, c0 + cw)
                        po = psum.tile([P, cw], f32, tag="po", bufs=4, name="po")
                        for fi in range(N_FI):
                            nc.tensor.matmul(
                                po[:],
                                w2_t[:, fi, bass.ts(ht, P)],
                                yt[:, fi, csl],
                                start=(fi == 0),
                                stop=(fi == N_FI - 1),
                            )
                        nc.vector.tensor_tensor(
                            ot[:, csl], po[:], gw_s[:, csl], mybir.AluOpType.mult
                        )
                        c0 += cw
                    nc.sync.dma_start(out_r[:, ht, o : o + L], ot[:])
    nc.finalize()
    return nc


def _route(x, gate_w):
    """Host gate: top-2 + softmax.  Returns (xs, per-expert idx, weights)."""
    xs = x.reshape(-1, x.shape[-1])
    logits = xs.astype(np.float32) @ gate_w.astype(np.float32)  # [T, E]
    e1 = np.argmax(logits, axis=1)
    l1 = logits[np.arange(len(logits)), e1]
    masked = logits.copy()
    masked[np.arange(len(logits)), e1] = -np.inf
    e2 = np.argmax(masked, axis=1)
    l2 = masked[np.arange(len(logits)), e2]
    w_hi = 1.0 / (1.0 + np.exp(l2 - l1))
    w_lo = 1.0 - w_hi
    idxs, gws = [], []
    for e in range(E):
        sel1 = e1 == e
        sel2 = e2 == e
        idx = np.nonzero(sel1 | sel2)[0]
        w = np.where(sel1[idx], w_hi[idx], w_lo[idx]).astype(np.float32)
        idxs.append(idx)
        gws.append(w)
    return xs, idxs, gws


def _slice_weights(w1, w3, w2):
    """Pre-arrange weight slices for all cores.

    w1/w3 -> [E, 64, 128, 2048] bf16 where [e, fg, hi, (ho f)] =
             w[e, ho*128+hi, fg*128+f]   (fg = global f-tile index)
    w2    -> [E, 64, 128, 2048] bf16 where [e, fg, hi, h] =
             w2[e, fg*128+hi, h]
    """
    bf16 = ml_dtypes.bfloat16
    w1a = np.ascontiguousarray(
        w1.reshape(E, N_HH, P, F // P, P).transpose(0, 3, 2, 1, 4)
    ).reshape(E, F // P, P, H).astype(bf16)
    w3a = np.ascontiguousarray(
        w3.reshape(E, N_HH, P, F // P, P).transpose(0, 3, 2, 1, 4)
    ).reshape(E, F // P, P, H).astype(bf16)
    w2a = w2.reshape(E, F // P, P, H).astype(bf16)
    return w1a, w3a, w2a


def kernel(x, gate_w, w1, w3, w2):
    x = np.asarray(x)
    gate_w = np.asarray(gate_w)
    w1 = np.asarray(w1)
    w3 = np.asarray(w3)
    w2 = np.asarray(w2)
    bf16 = ml_dtypes.bfloat16

    xs, idxs, gws = _route(x, gate_w)
    T = xs.shape[0]

    # Build segments: (expert, token index array, padded length).  Experts
    # with more than SEG_CAP tokens are split into multiple segments.
    segs = []
    for e in range(E):
        idx = idxs[e]
        gw_e = gws[e]
        for s0 in range(0, max(len(idx), 1), SEG_CAP):
            part = idx[s0 : s0 + SEG_CAP]
            if len(part) == 0:
                continue
            Lp = (len(part) + 7) // 8 * 8
            segs.append((e, part, gw_e[s0 : s0 + len(part)], Lp))

    seg_lens = tuple(Lp for _, _, _, Lp in segs)
    if seg_lens not in _kernel_cache:
        _kernel_cache[seg_lens] = build_kernel(list(seg_lens))
    nc = _kernel_cache[seg_lens]

    L_tot = sum(seg_lens)
    offs = np.concatenate([[0], np.cumsum(seg_lens)]).astype(int)

    # Shared inputs: token matrix (transposed, bf16) and replicated gates.
    xt = np.zeros((H, L_tot), bf16)
    gwb_row = np.zeros(L_tot, np.float32)
    for si, (e, part, gw_e, Lp) in enumerate(segs):
        o = int(offs[si])
        xt[:, o : o + len(part)] = xs[part].T.astype(bf16)
        gwb_row[o : o + len(part)] = gw_e
    gwb = np.ascontiguousarray(np.broadcast_to(gwb_row, (P, L_tot)))

    w1a, w3a, w2a = _slice_weights(w1, w3, w2)

    in_maps = []
    for c in range(N_CORES):
        fsl = slice(c * N_FI, (c + 1) * N_FI)
        w1c = np.ascontiguousarray(w1a[:, fsl]).reshape(E * N_FI * P, H)
        w3c = np.ascontiguousarray(w3a[:, fsl]).reshape(E * N_FI * P, H)
        w2c = np.ascontiguousarray(w2a[:, fsl]).reshape(E * N_FI * P, H)
        # reorder rows to segment order (handles split segments)
        if len(segs) != E or any(si != segs[si][0] for si in range(len(segs))):
            rows1, rows3, rows2 = [], [], []
            for e, _, _, _ in segs:
                sl = slice(e * N_FI * P, (e + 1) * N_FI * P)
                rows1.append(w1c[sl])
                rows3.append(w3c[sl])
                rows2.append(w2c[sl])
            w1c = np.concatenate(rows1, axis=0)
            w3c = np.concatenate(rows3, axis=0)
            w2c = np.concatenate(rows2, axis=0)
        in_maps.append(
            {"xt": xt, "gwb": gwb, "w1s": w1c, "w3s": w3c, "w2s": w2c}
        )

    global LAST_RESULT
    if TRACE:
        try:
            res = run_bass_kernel_spmd(
                nc,
                in_maps,
                core_ids=list(range(N_CORES)),
                trace=True,
                trace_cores=list(range(N_CORES)),
            )
        except Exception as exc:
            import traceback

            print("TRACE FAILED:", exc)
            traceback.print_exc()
            res = run_bass_kernel_spmd(nc, in_maps, core_ids=list(range(N_CORES)))
    else:
        res = run_bass_kernel_spmd(nc, in_maps, core_ids=list(range(N_CORES)))
    LAST_RESULT = res

    out_sum = np.zeros((H, L_tot), np.float32)
    for c in range(N_CORES):
        out_sum += res.results[c]["out"]

    out_flat = np.zeros((T, H), np.float32)
    for si, (e, part, gw_e, Lp) in enumerate(segs):
        o = int(offs[si])
        out_flat[part] += out_sum[:, o : o + len(part)].T
    return out_flat.reshape(x.shape).astype(x.dtype)
